# revision 39
# baseline (speedup 1.0000x reference)
"""Trainium2 Bass kernel for nn_LocalMambaBlock (self-contained).

Sharding: 8 cores = 4 batches x 2 d_inner halves. Each core (b, j) computes
u = silu(causal_conv(x[b] @ W_in_u)) for its d_inner half, pair-AllReduces
the partial x_proj, runs the selective scan over its 1024 channels x 16
states, gates with silu(z), and emits a partial out-projection the host sums.

Engine assignment (all verified against the REAL compiler, not just the
cost model — GPSIMD cannot run TensorScalarPtr ops (scan/scalar_tensor_
tensor/tensor_scalar) and cannot touch PSUM; only TensorTensor/TensorCopy/
Memset/custom-ISA are legal there):
  DVE : all 256 tensor_tensor_scans (engine-pinned), duB multiplies with a
        3-n software-pipelined lookahead (emission order matters: an
        in-order engine queue must never interleave a scan-dependent op
        before an independent one), du/gate/ygh, conv taps, proj evac.
  POOL: 13 of 16 h*C multiplies per unit (plain TensorTensor), collective.
  ACT : all dA = exp(A_n*delta) (scale=A_n fused), softplus via in-place
        Exp+Ln batches (Exp and Ln end up in different act-table sets:
        interleaving them costs a 1.28us LoadActFuncSet per transition, so
        phases are batched per half), silu(u/z), carry copies ([128,1] is
        ~free on ACT), PSUM evacs.
  PE  : u/z/x_proj/dt/out-proj matmuls + identity-matmul accumulation of
        y += h_n*C_n into PSUM; z matmuls folded into phase A sharing the
        pu PSUM tag; h1's z matmuls spread into h0's units.
DMA: everything rides the SP queue except wait-free input loads (ACT queue)
and fp32->bf16 cast DMAs for dtr/bmc via the gpsimd SWDGE queue. B/C rows
are broadcast in 2-state groups from an interleaved [n][B|C][t] DRAM
layout (fewer HWDGE holds: each DMA costs ~630ns on the single shared
HWDGE device and blocks its issuing queue while its deps resolve — never
queue a waiting DMA ahead of compute on ACT).

Known-good pitfalls baked in: tile-pool closes emit boundaries that stall
every engine queue until the pool's last consumer finishes (keep pool-exit
consumers early); PSUM tags get bufs buffers each (2 tags x bufs=2 of
[128,1024]fp32 = 8 banks = all of PSUM); WAR on a shared tile tag across
an in-order queue deadlocks if the later writer precedes the earlier
reader in queue order.

Round 2: Dp*u folded into the PSUM y-accumulation as an extra matmul with
a host-built block-diagonal diag(Dp) stationary (kills the ygh stt and
fuses the gate into one PSUM-reading multiply, and improves precision:
the +Dp*u add now happens in fp32 PSUM); carry copies emitted 2 n's late
so they never make the ACT queue wait on a live scan; h1 out-proj group 0
starts its k-accumulation before the last gates (split start/stop matmul
bursts into the same PSUM tile). SBUF is at the wall: opool bufs=2 and
LOOK=4 both overflow.

Round 3: phase A split by t-half — x_proj only needs u's columns per
half, so xproj(h0)+AllReduce(h0) fire after half the u matmuls and the
whole AR chain, z matmuls, B/C head broadcasts (first 2 groups from a
program-scope pool) and delta prep overlap the h1 u/conv work. Two
AllReduces on a half-major [2, 96, TH] proj layout. Head serial depth
~150us -> ~130us; remaining head floor = PE's ~95us of matmul work plus
the DVE conv(h1) chain (fp8 DoubleRow or PE-diag conv would be next).

Round 4-5: gate split — ACT evacuates py (PSUM fp32) into the dead u
slice, then the gate is an in-place bf16 SBUF multiply, keeping DVE's 2x
mode (a PSUM/fp32 operand halves DVE tensor_tensor throughput); all h0
out-proj evacs moved to ACT for the same reason. fp8 DoubleRow for the
u/z projections was built and VERIFIED mechanically (712us in sim, 4x PE
throughput) but FAILED precision (4.58e-2 vs the 2e-2 gate: quantization
noise and signal both grow as sqrt(K) through the reduction, so e4m3's
~5%/element noise survives at full strength) and was reverted.

Round 6: output partials in bf16 (host sums the two cores' partials in
fp32) — halves the output DMA and shrinks osb tiles enough to double-
buffer them inside the SBUF wall, collapsing the tail's evac/DMA
ping-pong. Tail duB/hc re-assignment probes (DBP=1/2 on Pool) regressed;
steady-state DVE is saturated at 93-100% — only work reduction moves it.

Round 7: h1 delta sub-batch ([0,1] at the boundary, rest emitted after
unit 0) mirrors h0's pattern and removes ~7µs of boundary ACT-serial
wait. Sub-batching the zsil1 silus the same way DEADLOCKS (pz1 PSUM
slots need their silu consumers within 2 allocations).

TimelineSim estimate 723.6us/core (baseline 805us); rel err 7.10e-3 on
the 8-core fake_nrt run.
"""
import sys

sys.path.insert(0, "/opt/trn_rl_repo")

import numpy as np
import ml_dtypes

BF = ml_dtypes.bfloat16

B, L, DM = 4, 2048, 1024
DI = 2048
DH = DI // 2
NST = 16
R = 64
KC = 4
NCORES = 8
TH = L // 2

NPOOL = 12          # h*C mults per unit on GPSIMD; rest on DVE
DBP = 0             # tail duB mults per unit on GPSIMD (0: all DVE)

_prog_cache = {}


def _build_program(sim_mode=False):
    import concourse.bacc as bacc
    import concourse.tile as tile
    from concourse import mybir

    FP32 = mybir.dt.float32
    BF16 = mybir.dt.bfloat16
    MULT = mybir.AluOpType.mult
    ADD = mybir.AluOpType.add
    AF = mybir.ActivationFunctionType

    from concourse.bass import _add_dep_helper

    def _add_dep(a, b):
        _add_dep_helper(a, b, sync=True, reason="act-table phase ordering")

    nc = bacc.Bacc(None)

    xT = nc.dram_tensor("xT", [DM, L], BF16, kind="ExternalInput")
    wu = nc.dram_tensor("wu", [DM, DH], BF16, kind="ExternalInput")
    wz = nc.dram_tensor("wz", [DM, DH], BF16, kind="ExternalInput")
    wxp = nc.dram_tensor("wxp", [DH, R + 2 * NST], BF16, kind="ExternalInput")
    wdt = nc.dram_tensor("wdt", [R, DH], BF16, kind="ExternalInput")
    consts = nc.dram_tensor("consts", [DH, KC + 3 + NST], FP32, kind="ExternalInput")
    wo = nc.dram_tensor("wo", [DH, DM], BF16, kind="ExternalInput")
    ident = nc.dram_tensor("ident", [128, 128], BF16, kind="ExternalInput")
    dpd = nc.dram_tensor("dpd", [128, DH], BF16, kind="ExternalInput")

    outT = nc.dram_tensor("outT", [DM, L], BF16, kind="ExternalOutput")

    proj_src = nc.dram_tensor("proj_src", [2, R + 2 * NST, TH], FP32)
    proj_dst = nc.dram_tensor("proj_dst", [2, R + 2 * NST, TH], FP32)
    # interleaved [n][B|C][t] so one DMA broadcasts a 4-n group of B and C
    bmc_dram = nc.dram_tensor("bmc_dram", [NST, 2, L], BF16)

    NDT = DH // 128
    NK = DM // 128
    NM = DM // 128

    with tile.TileContext(nc) as tc:
        import contextlib
        es = contextlib.ExitStack()
        with es:
            persist = es.enter_context(tc.tile_pool(name="persist", bufs=1))
            wxp_t = []

            NCC = KC + 3 + NST
            cst_t = []
            for i in range(NDT):
                t = persist.tile([128, NCC], FP32, tag=f"cst{i}")
                nc.scalar.dma_start(t[:], consts[i * 128:(i + 1) * 128, :])
                cst_t.append(t)
            cw_t = [c[:, 0:KC] for c in cst_t]
            cb_t = [c[:, KC:KC + 1] for c in cst_t]
            dp_t = [c[:, KC + 1:KC + 2] for c in cst_t]
            bdt_t = [c[:, KC + 2:KC + 3] for c in cst_t]
            at_t = [c[:, KC + 3:KC + 3 + NST] for c in cst_t]
            id_t = persist.tile([128, 128], BF16, tag="ident")
            nc.scalar.dma_start(id_t[:], ident[:])
            dpd_t = persist.tile([128, DH], BF16, tag="dpd")
            nc.scalar.dma_start(dpd_t[:], dpd[:])
            wdt_all = persist.tile([R, DH], BF16, tag="wdt_all")
            nc.scalar.dma_start(wdt_all[:], wdt[:])
            wdt_t = [wdt_all[:, i * 128:(i + 1) * 128] for i in range(NDT)]
            dtr = persist.tile([R, L], BF16, tag="dtr")
            carry = []
            for i in range(NDT):
                ct = persist.tile([128, NST], BF16, tag=f"carry{i}")
                carry.append(ct)
            u_t = []
            for i in range(NDT):
                ui = persist.tile([128, L], BF16, tag=f"u{i}")
                u_t.append(ui)
            wz_t = []

            # ---------- phase A: u (own half) + partial x_proj ----------
            xhpool = es.enter_context(tc.tile_pool(name="xhpool", bufs=1))
            zhpool = es.enter_context(tc.tile_pool(name="zhpool", bufs=1))
            bcapool = es.enter_context(tc.tile_pool(name="bcapool", bufs=1))

            def emit_bc_group(g, th, pool):
                t0 = th * TH
                t = pool.tile([128, 4 * TH], BF16, tag=f"bcg{g}")
                nc.sync.dma_start(
                    t[:],
                    bmc_dram[2 * g:2 * (g + 1), :,
                             t0:t0 + TH].partition_broadcast(128))
                out = []
                for r in range(2):
                    out.append((t[:, (2 * r) * TH:(2 * r + 1) * TH],
                                t[:, (2 * r + 1) * TH:(2 * r + 2) * TH]))
                return out

            def emit_xh_loads(th):
                t0 = th * TH
                xh_t = []
                for k in range(NK):
                    t = xhpool.tile([128, TH], BF16, tag=f"xh{k}")
                    nc.sync.dma_start(t[:], xT[k * 128:(k + 1) * 128,
                                               t0:t0 + TH])
                    xh_t.append(t)
                return xh_t

            with tc.tile_pool(name="xzscope", bufs=1) as xpool, \
                 tc.tile_pool(name="upool", bufs=1) as upool, \
                 tc.tile_pool(name="cpool", bufs=2) as cpool, \
                 tc.tile_pool(name="psum_mm", bufs=2, space="PSUM") as psum_mm, \
                 tc.tile_pool(name="psum_proj", bufs=1, space="PSUM") as psum_proj:
                xt_t = []
                wu_t = []
                for k in range(NK):
                    t = xpool.tile([128, L], BF16, tag=f"xt{k}")
                    nc.scalar.dma_start(t[:], xT[k * 128:(k + 1) * 128, :])
                    xt_t.append(t)
                    w = xpool.tile([128, DH], BF16, tag=f"wuk{k}")
                    nc.scalar.dma_start(w[:], wu[k * 128:(k + 1) * 128, :])
                    wu_t.append(w)
                # z weights resident for the scan-phase z matmuls
                for k in range(NK):
                    w = persist.tile([128, DH], BF16, tag=f"wzk{k}")
                    nc.scalar.dma_start(w[:], wz[k * 128:(k + 1) * 128, :])
                    wz_t.append(w)

                z0 = []
                zs0_ins = []
                upre_t = []
                for i in range(NDT):
                    upre = upool.tile([128, L + KC - 1], BF16, tag=f"upre{i}")
                    nc.vector.memset(upre[:, 0:KC - 1], 0.0)
                    upre_t.append(upre)

                def emit_u_half(hh):
                    for i in range(NDT):
                        upre = upre_t[i]
                        pu = psum_mm.tile([128, TH], FP32, tag="pu")
                        for k in range(NK):
                            for c4 in range(TH // 512):
                                nc.tensor.matmul(
                                    pu[:, c4 * 512:(c4 + 1) * 512],
                                    wu_t[k][:, i * 128:(i + 1) * 128],
                                    xt_t[k][:, hh * TH + c4 * 512:
                                             hh * TH + (c4 + 1) * 512],
                                    start=(k == 0), stop=(k == NK - 1))
                        nc.scalar.copy(
                            upre[:, KC - 1 + hh * TH:KC - 1 + (hh + 1) * TH],
                            pu[:])
                        c_a = cpool.tile([128, TH], BF16, tag="cacc0")
                        nc.vector.tensor_scalar_mul(
                            c_a[:], upre[:, hh * TH:hh * TH + TH],
                            cw_t[i][:, 0:1])
                        for kk in range(1, KC):
                            c_b = cpool.tile([128, TH], BF16,
                                             tag=f"cacc{kk % 2}")
                            nc.vector.scalar_tensor_tensor(
                                c_b[:], upre[:, hh * TH + kk:hh * TH + kk + TH],
                                cw_t[i][:, kk:kk + 1], c_a[:],
                                op0=MULT, op1=ADD)
                            c_a = c_b
                        ls = nc.scalar.activation(
                            u_t[i][:, hh * TH:(hh + 1) * TH], c_a[:],
                            AF.Silu, bias=cb_t[i])
                        if hh == 0:
                            wx = xpool.tile([128, R + 2 * NST], BF16,
                                            tag=f"wxp{i}")
                            nc.sync.dma_start(
                                wx[:], wxp[i * 128:(i + 1) * 128, :])
                            wxp_t.append(wx)
                    return ls

                def emit_xproj_ar(hh):
                    t0 = hh * TH
                    pp = psum_proj.tile([R + 2 * NST, TH], FP32, tag="pproj")
                    for i in range(NDT):
                        for c4 in range(TH // 512):
                            nc.tensor.matmul(
                                pp[:, c4 * 512:(c4 + 1) * 512], wxp_t[i][:],
                                u_t[i][:, t0 + c4 * 512:t0 + (c4 + 1) * 512],
                                start=(i == 0), stop=(i == NDT - 1))
                    projx = upool.tile([R + 2 * NST, TH], FP32,
                                       tag=f"projx{hh}")
                    nc.vector.tensor_copy(projx[:], pp[:])
                    nc.sync.dma_start(proj_src[hh], projx[:])
                    if sim_mode:
                        nc.sync.dma_start(proj_dst[hh], proj_src[hh])
                    else:
                        nc.gpsimd.collective_compute(
                            "AllReduce", mybir.AluOpType.add,
                            replica_groups=[[0, 1], [2, 3], [4, 5], [6, 7]],
                            ins=[proj_src[hh]], outs=[proj_dst[hh]])
                    nc.gpsimd.dma_start(dtr[:, t0:t0 + TH],
                                        proj_dst[hh, 0:R, :])
                    nc.gpsimd.dma_start(bmc_dram[:, 0, t0:t0 + TH],
                                        proj_dst[hh, R:R + NST, :])
                    nc.gpsimd.dma_start(bmc_dram[:, 1, t0:t0 + TH],
                                        proj_dst[hh, R + NST:R + 2 * NST, :])

                emit_u_half(0)
                emit_xproj_ar(0)
                bc_head = [emit_bc_group(g, 0, bcapool) for g in range(2)]
                # z matmuls + silus for h0 run during the h0 AllReduce
                for i in range(NDT):
                    pz = psum_mm.tile([128, TH], FP32, tag="pu")
                    for k in range(NK):
                        for c4 in range(TH // 512):
                            nc.tensor.matmul(
                                pz[:, c4 * 512:(c4 + 1) * 512],
                                wz_t[k][:, i * 128:(i + 1) * 128],
                                xt_t[k][:, c4 * 512:(c4 + 1) * 512],
                                start=(k == 0), stop=(k == NK - 1))
                    zh = zhpool.tile([128, TH], BF16, tag=f"zh{i}")
                    zs0_ins.append(nc.scalar.activation(zh[:], pz[:], AF.Silu))
                    z0.append(zh)
                last_silu = emit_u_half(1)
                emit_xproj_ar(1)

            # ---------- scan phase: two t-halves ----------
            opool = es.enter_context(tc.tile_pool(name="opool", bufs=2))
            wopool = es.enter_context(tc.tile_pool(name="wopool", bufs=1))
            with tc.tile_pool(name="bcpool", bufs=1) as bcpool, \
                 tc.tile_pool(name="spool", bufs=3) as spool, \
                 tc.tile_pool(name="dpool", bufs=1) as dpool, \
                 tc.tile_pool(name="dbpool", bufs=4) as dbpool, \
                 tc.tile_pool(name="dlpool", bufs=1) as dlpool, \
                 tc.tile_pool(name="psum_y", bufs=2, space="PSUM") as psum_y, \
                 tc.tile_pool(name="psum_po", bufs=2, space="PSUM") as psum_po:
                def emit_bc_loads(th, skip_head=False):
                    b_bc, c_bc = [], []
                    for g in range(NST // 2):
                        if th == 0 and skip_head and g < 2:
                            pairs = bc_head[g]
                        else:
                            pool = bcapool if g < 2 else bcpool
                            pairs = emit_bc_group(g, th, pool)
                        for b, c in pairs:
                            b_bc.append(b)
                            c_bc.append(c)
                    return b_bc, c_bc

                state = {"last_da": None}

                def emit_delta(th, zsilu_ins, subset):
                    t0 = th * TH
                    deltas, exp_ins, ln_ins = [], [], []
                    for i in subset:
                        pd = psum_po.tile([128, TH], FP32, tag="mm")
                        for c4 in range(TH // 512):
                            nc.tensor.matmul(
                                pd[:, c4 * 512:(c4 + 1) * 512], wdt_t[i],
                                dtr[:, t0 + c4 * 512:t0 + (c4 + 1) * 512],
                                start=True, stop=True)
                        delta = dlpool.tile([128, TH], BF16, tag=f"delta{i}")
                        e_ins = nc.scalar.activation(delta[:], pd[:], AF.Exp,
                                                     bias=bdt_t[i])
                        if zsilu_ins:
                            _add_dep(e_ins.ins, zsilu_ins[-1].ins)
                        elif state["last_da"] is not None:
                            _add_dep(e_ins.ins, state["last_da"].ins)
                        deltas.append(delta)
                        exp_ins.append(e_ins)
                    for d in deltas:
                        l_ins = nc.scalar.activation(d[:], d[:],
                                                     AF.Ln, bias=1.0)
                        _add_dep(l_ins.ins, exp_ins[-1].ins)
                        ln_ins.append(l_ins)
                    return deltas, ln_ins

                def emit_pd(th):
                    t0 = th * TH
                    pd_t = []
                    for i in range(NDT):
                        pd = psum_po.tile([128, TH], FP32, tag="mm")
                        for c4 in range(TH // 512):
                            nc.tensor.matmul(
                                pd[:, c4 * 512:(c4 + 1) * 512], wdt_t[i],
                                dtr[:, t0 + c4 * 512:t0 + (c4 + 1) * 512],
                                start=True, stop=True)
                        pd_t.append(pd)
                    return pd_t

                def emit_delta_from_pd(th, pd_t):
                    deltas, exp_ins, ln_ins = [], [], []
                    for i in range(NDT):
                        delta = dlpool.tile([128, TH], BF16, tag=f"delta{i}")
                        e_ins = nc.scalar.activation(delta[:], pd_t[i][:],
                                                     AF.Exp, bias=bdt_t[i])
                        if state["last_da"] is not None:
                            _add_dep(e_ins.ins, state["last_da"].ins)
                        deltas.append(delta)
                        exp_ins.append(e_ins)
                    for d in deltas:
                        l_ins = nc.scalar.activation(d[:], d[:],
                                                     AF.Ln, bias=1.0)
                        _add_dep(l_ins.ins, exp_ins[-1].ins)
                        ln_ins.append(l_ins)
                    return deltas, ln_ins

                LOOK = 3

                def emit_du(th, i, deltas):
                    t0 = th * TH
                    du = dpool.tile([128, TH], BF16, tag=f"du{i % 2}")
                    nc.vector.tensor_tensor(du[:], deltas[i][:],
                                            u_t[i][:, t0:t0 + TH], op=MULT)
                    return du

                def emit_duB(i, n, du, b_bc):
                    duB = dbpool.tile([128, TH], BF16, tag="duB")
                    deng = nc.gpsimd if n >= NST - DBP else nc.vector
                    deng.tensor_tensor(duB[:], du[:], b_bc[n][:], op=MULT)
                    return duB

                def emit_unit(th, i, deltas, b_bc, c_bc, z_h, ln_ins,
                              pre, nxt_pre):
                    """pre: (du, [duB_0..LOOK-1]) for THIS unit; nxt_pre()
                    emits the next unit's prologue mid-tail and returns it."""
                    t0 = th * TH
                    du, duBs = pre
                    py = psum_y.tile([128, TH], FP32, tag="py")
                    ret = None
                    h_hist = {}
                    for n in range(NST):
                        dA = spool.tile([128, TH], BF16, tag="dA")
                        da_ins = nc.scalar.activation(
                            dA[:], deltas[i][:], AF.Exp,
                            scale=at_t[i][:, n:n + 1])
                        if n == 0:
                            _add_dep(da_ins.ins, ln_ins[-1].ins)
                        state["last_da"] = da_ins
                        h = spool.tile([128, TH], BF16, tag="h")
                        init = 0.0 if th == 0 else carry[i][:, n:n + 1]
                        nc.vector.tensor_tensor_scan(h[:], dA[:], duBs[n][:],
                                                     init, op0=MULT, op1=ADD)
                        h_hist[n] = h
                        # carry copy delayed 2 n's so it never waits a live scan
                        if th == 0 and n >= 2:
                            nc.scalar.copy(carry[i][:, n - 2:n - 1],
                                           h_hist[n - 2][:, TH - 1:TH])
                        hc = spool.tile([128, TH], BF16, tag="hc")
                        heng = nc.gpsimd if n < NPOOL else nc.vector
                        heng.tensor_tensor(hc[:], h[:], c_bc[n][:], op=MULT)
                        if n + LOOK < NST:
                            duBs.append(emit_duB(i, n + LOOK, du, b_bc))
                        elif n == NST - LOOK and nxt_pre is not None:
                            ret = nxt_pre()
                        for c4 in range(TH // 512):
                            nc.tensor.matmul(
                                py[:, c4 * 512:(c4 + 1) * 512], id_t[:],
                                hc[:, c4 * 512:(c4 + 1) * 512],
                                start=(n == 0), stop=False)
                    if th == 0:
                        for n in (NST - 2, NST - 1):
                            nc.scalar.copy(carry[i][:, n:n + 1],
                                           h_hist[n][:, TH - 1:TH])
                    # y += Dp*u via block-diagonal weights (replaces ygh)
                    for c4 in range(TH // 512):
                        nc.tensor.matmul(
                            py[:, c4 * 512:(c4 + 1) * 512],
                            dpd_t[:, i * 128:(i + 1) * 128],
                            u_t[i][:, t0 + c4 * 512:t0 + (c4 + 1) * 512],
                            start=False, stop=(c4 == TH // 512 - 1))
                    # evac y into the (now-dead) u slice on ACT, then gate
                    # as an in-place bf16 SBUF multiply (keeps DVE 2x mode)
                    nc.scalar.copy(u_t[i][:, t0:t0 + TH], py[:])
                    nc.vector.tensor_tensor(u_t[i][:, t0:t0 + TH],
                                            u_t[i][:, t0:t0 + TH],
                                            z_h[i][:], op=MULT)
                    return ret

                def emit_wok_loads(mg, ks):
                    for k in ks:
                        wok = wopool.tile([128, 256], BF16, tag=f"wok{k}")
                        wok_t[k] = wok
                        nc.sync.dma_start(
                            wok[:], wo[k * 128:(k + 1) * 128,
                                       mg * 256:(mg + 1) * 256])

                def emit_outproj_group(th, mg, evac, ks=None, final=True,
                                       first=True, loads=True, osb_q=None):
                    t0 = th * TH
                    ks = list(range(NDT)) if ks is None else ks
                    if loads:
                        emit_wok_loads(mg, ks)
                    for mh in range(2):
                        m = 2 * mg + mh
                        if first:
                            po = psum_po.tile([128, TH], FP32, tag="mm")
                            po_t[mh] = po
                        po = po_t[mh]
                        for k in ks:
                            for c4 in range(TH // 512):
                                nc.tensor.matmul(
                                    po[:, c4 * 512:(c4 + 1) * 512],
                                    wok_t[k][:, mh * 128:(mh + 1) * 128],
                                    u_t[k][:, t0 + c4 * 512:t0 + (c4 + 1) * 512],
                                    start=(first and k == ks[0]),
                                    stop=(final and k == ks[-1]))
                        if final:
                            osb = opool.tile([128, TH], BF16, tag="osb")
                            if evac == "act":
                                nc.scalar.copy(osb[:], po[:])
                            else:
                                nc.vector.tensor_copy(osb[:], po[:])
                            q = osb_q or nc.sync
                            q.dma_start(
                                outT[m * 128:(m + 1) * 128, t0:t0 + TH], osb[:])

                wok_t = {}
                po_t = {}

                def emit_z_mm_one(th, i, xh_t):
                    t0 = th * TH
                    pz = psum_po.tile([128, TH], FP32, tag="mm")
                    for k in range(NK):
                        for c4 in range(TH // 512):
                            nc.tensor.matmul(
                                pz[:, c4 * 512:(c4 + 1) * 512],
                                wz_t[k][:, i * 128:(i + 1) * 128],
                                xh_t[k][:, c4 * 512:(c4 + 1) * 512],
                                start=(k == 0), stop=(k == NK - 1))
                    return pz

                def emit_z_silu_one(i, pz):
                    zh = zhpool.tile([128, TH], BF16, tag=f"zh{i}")
                    zs = nc.scalar.activation(zh[:], pz[:], AF.Silu)
                    if state["last_da"] is not None:
                        _add_dep(zs.ins, state["last_da"].ins)
                    return zh, zs

                # ---- half 0 ----
                b0, c0 = emit_bc_loads(0, skip_head=True)
                d0, ln0 = emit_delta(0, zs0_ins, [0, 1])
                xh1 = None
                bc1 = None
                pz1 = []
                zsil1 = {}
                pd1 = []
                def mk_pre(th, j, dl, bb):
                    def f():
                        du = emit_du(th, j, dl)
                        return (du, [emit_duB(j, n, du, bb)
                                     for n in range(LOOK)])
                    return f

                pre = mk_pre(0, 0, d0, b0)()
                for i in range(NDT):
                    nxt = mk_pre(0, i + 1, d0, b0) if i + 1 < NDT else None
                    pre = emit_unit(0, i, d0, b0, c0, z0, ln0, pre, nxt)
                    if i == 0:
                        d0b, ln0 = emit_delta(0, zs0_ins, list(range(2, NDT)))
                        d0.extend(d0b)
                        xh1 = emit_xh_loads(1)
                        bc1 = emit_bc_loads(1)
                    if 2 <= i <= 5:
                        j = 2 * (i - 2)
                        pz1.append(emit_z_mm_one(1, j, xh1))
                        pz1.append(emit_z_mm_one(1, j + 1, xh1))
                        if 3 <= i <= 5:
                            j = 2 * (i - 3)
                            zsil1[j] = emit_z_silu_one(j, pz1[j])
                            zsil1[j + 1] = emit_z_silu_one(j + 1, pz1[j + 1])
                    if i == 6:
                        for j in range(6, NDT):
                            zsil1[j] = emit_z_silu_one(j, pz1[j])
                # ---- half 1 ----
                b1, c1 = bc1
                z1 = [zsil1[i][0] for i in range(NDT)]
                d1, ln1 = emit_delta(1, [], [0, 1])
                pre = mk_pre(1, 0, d1, b1)()
                for i in range(NDT):
                    nxt = mk_pre(1, i + 1, d1, b1) if i + 1 < NDT else None
                    pre = emit_unit(1, i, d1, b1, c1, z1, ln1, pre, nxt)
                    if i == 0:
                        d1b, ln1 = emit_delta(1, [], list(range(2, NDT)))
                        d1.extend(d1b)
                    if i < NM // 2:
                        emit_outproj_group(0, i, "act")
                    if i == 5:
                        # start h1 out-proj mg0: gates 0..4 are final
                        emit_outproj_group(1, 0, "act", ks=list(range(5)),
                                           final=False, first=True)
                    if i == 6:
                        emit_outproj_group(1, 0, "act", ks=[5],
                                           final=False, first=False)
                emit_outproj_group(1, 0, "act", ks=[6, 7], final=True,
                                   first=False, osb_q=nc.scalar)
                emit_wok_loads(1, list(range(NDT)))
                for mg in range(1, NM // 2):
                    if mg + 1 < NM // 2:
                        emit_wok_loads(mg + 1, list(range(NDT)))
                    emit_outproj_group(1, mg, "act", loads=False,
                                       osb_q=nc.scalar)

    nc.finalize()
    return nc


def _get_program():
    if "nc" not in _prog_cache:
        _prog_cache["nc"] = _build_program()
    return _prog_cache["nc"]


def kernel(**inputs):
    from concourse.bass_utils import run_bass_kernel_spmd

    x = np.asarray(inputs["x"], np.float32)
    W_in = np.asarray(inputs["W_in"], np.float32)
    conv_w = np.asarray(inputs["conv_w"], np.float32)
    conv_b = np.asarray(inputs["conv_b"], np.float32)
    W_xproj = np.asarray(inputs["W_xproj"], np.float32)
    W_dt = np.asarray(inputs["W_dt"], np.float32)
    b_dt = np.asarray(inputs["b_dt"], np.float32)
    A_log = np.asarray(inputs["A_log"], np.float32)
    Dp = np.asarray(inputs["Dp"], np.float32)
    W_out = np.asarray(inputs["W_out"], np.float32)

    aneg_full = -np.exp(A_log)
    ident = np.eye(128, dtype=BF)
    consts_full = np.concatenate([
        conv_w, conv_b[:, None], Dp[:, None], b_dt[:, None], aneg_full,
    ], axis=1).astype(np.float32)

    half = []
    for j in range(2):
        ds = slice(j * DH, (j + 1) * DH)
        dph = Dp[ds]
        dpd_h = np.zeros((128, DH), dtype=BF)
        for i in range(DH // 128):
            dpd_h[:, i * 128:(i + 1) * 128] = np.diag(
                dph[i * 128:(i + 1) * 128]).astype(BF)
        half.append({
            "dpd": dpd_h,
            "wu": np.ascontiguousarray(W_in[:, ds]).astype(BF),
            "wz": np.ascontiguousarray(
                W_in[:, DI + j * DH:DI + (j + 1) * DH]).astype(BF),
            "consts": np.ascontiguousarray(consts_full[ds]),
            "wxp": np.ascontiguousarray(W_xproj[ds]).astype(BF),
            "wdt": np.ascontiguousarray(W_dt[:, ds]).astype(BF),
            "wo": np.ascontiguousarray(W_out[ds]).astype(BF),
            "ident": ident,
        })
    xTs = [np.ascontiguousarray(x[b].T).astype(BF) for b in range(B)]

    in_maps = []
    for core in range(NCORES):
        b, j = core // 2, core % 2
        m = dict(half[j])
        m["xT"] = xTs[b]
        in_maps.append(m)

    nc = _get_program()
    res = run_bass_kernel_spmd(nc, in_maps, core_ids=list(range(NCORES)))
    out = np.empty((B, L, DM), np.float32)
    for b in range(B):
        o = (res.results[2 * b]["outT"].astype(np.float32) +
             res.results[2 * b + 1]["outT"].astype(np.float32))
        out[b] = o.T
    return out


if __name__ == "__main__":
    rng = np.random.default_rng(0)
    ins = {
        "x": rng.standard_normal((B, L, DM), dtype=np.float32),
        "W_in": rng.standard_normal((DM, 2 * DI), dtype=np.float32) * 0.02,
        "conv_w": rng.standard_normal((DI, KC), dtype=np.float32) * 0.2,
        "conv_b": np.zeros(DI, np.float32),
        "W_xproj": rng.standard_normal((DI, R + 2 * NST), dtype=np.float32) * 0.02,
        "W_dt": rng.standard_normal((R, DI), dtype=np.float32) * 0.02,
        "b_dt": rng.uniform(-4.0, -2.0, DI).astype(np.float32),
        "A_log": np.log(np.broadcast_to(np.arange(1, NST + 1, dtype=np.float32),
                                        (DI, NST))).copy(),
        "Dp": np.ones(DI, np.float32),
        "W_out": rng.standard_normal((DI, DM), dtype=np.float32) * 0.02,
    }
    o = kernel(**ins)
    print("kernel ran, out shape", o.shape, "absmax", np.abs(o).max())
